# revision 32
# baseline (speedup 1.0000x reference)
"""MinGRU layer (LN -> gate/candidate Linear -> minGRU scan -> residual) on 8 trn2 cores.

Problem (hardcoded): x [B=4, T=4096, H=1024] fp32, weights Wg/Wc [1024,1024],
biases bg/bc [1024], LN gamma/beta [1024].

Sharding: core c = (batch b = c//2, output-half p = c%2). Each core receives the
full transposed batch row (H on partitions, T on free) and computes z/c for its
512 output channels over all T; the minGRU recurrence is elementwise over (b,h)
so no collectives are needed.

Per 512-col chunk, built around fp8 DoubleRow matmuls (each streams a PAIR of
k-tiles -> 2x bf16 GEMM throughput; weights pre-scaled by 32 on host so fp8
stays in normal range):
  1. GEMM on RAW fp8 x (no prescale): P' = 32W.x8 over 4 k-pair DR matmuls plus
     one rank-1 aug DR matmul carrying both LayerNorm corrections:
     slot0 = (-4*wsumq) x (8*mu), slot1 = (32*b_eff) x (1/rstd).  Then
     tmp = P' * (rstd/32 broadcast) = W.((x-mu)*rstd) + b exactly (post-scale
     keeps P' in fp32 PSUM - no fp8 requantize of the scaled activations).
  2. Stats by the same DR trick: ones-weights (col 0 / col 1) sum x8 and x8^2
     pairs into rows 0/1 of one PSUM bank; squares x8^2 on ScalarE.
  3. rstd via 2 Newton steps for 1/sqrt(var+eps) on GpSimd in a DMA-transposed
     [128, 2, 4] layout (t on partitions, both chunks of a pair at once) - no
     Ln/Exp, so the ACT table never leaves the sigmoid/square/copy set.
  4. z = sigmoid(tmp_g) on ScalarE (bias pre-folded via aug slot1); zbar = 1-z
     (VectorE 4x); bsc = tmp_c*z (GpSimd); h = scan(zbar, bsc) (VectorE, the
     only engine with tensor_tensor_scan); residual h + x(fp16) on VectorE in
     2x mode; fp16 output, host transposes back.
"""

import os
import numpy as np
import ml_dtypes

import concourse.bass as bass
import concourse.bacc as bacc
import concourse.tile as tile
from concourse import mybir
from concourse.bass_utils import run_bass_kernel_spmd

B, T, H = 4, 4096, 1024
EPS = 1e-5
N_CORES = 8
OH = H // 2          # output channels per core
CHUNK = 512
N_CHUNKS = T // CHUNK
KT = H // 128        # k-tiles (contraction)
OT = OH // 128       # o-tiles per core
NPAIR = N_CHUNKS // 2

F32 = mybir.dt.float32
BF16 = mybir.dt.bfloat16
FP16 = mybir.dt.float16
FP8 = mybir.dt.float8e4
AF = mybir.ActivationFunctionType
OP = mybir.AluOpType
DR = mybir.MatmulPerfMode.DoubleRow
BF = ml_dtypes.bfloat16
F8 = ml_dtypes.float8_e4m3

_CACHE = {}


def _build():
    nc = bacc.Bacc("TRN2", target_bir_lowering=False, debug=False)

    x8_d = nc.dram_tensor("x8", [N_CHUNKS, 128, KT, CHUNK], FP8, kind="ExternalInput").ap()
    xr_d = nc.dram_tensor("xr", [N_CHUNKS, 128, OT, CHUNK], FP16, kind="ExternalInput").ap()
    wg_d = nc.dram_tensor("wg", [128, KT // 2, OT, 2, 128], FP8, kind="ExternalInput").ap()
    wc_d = nc.dram_tensor("wc", [128, KT // 2, OT, 2, 128], FP8, kind="ExternalInput").ap()
    augg_d = nc.dram_tensor("augg", [1, OT, 2, 128], FP8, kind="ExternalInput").ap()
    augc_d = nc.dram_tensor("augc", [1, OT, 2, 128], FP8, kind="ExternalInput").ap()
    onx_d = nc.dram_tensor("onx", [128, 2, 128], FP8, kind="ExternalInput").ap()
    onq_d = nc.dram_tensor("onq", [128, 2, 128], FP8, kind="ExternalInput").ap()
    out_d = nc.dram_tensor("outT", [N_CHUNKS, OT, 128, CHUNK], FP16, kind="ExternalOutput").ap()

    # Newton scratch in DRAM (partition-crossing transposes go through HBM).
    # stg rows per pair: (sx0, sq0, sx1, sq1)
    st_dram = nc.dram_tensor("st_sc", [NPAIR, 4, CHUNK], F32, kind="Internal").ap()
    rs_dram = nc.dram_tensor("rs_sc", [NPAIR, 2, CHUNK], BF16, kind="Internal").ap()
    ag_dram = nc.dram_tensor("ag_sc", [NPAIR, 2, 2, CHUNK], FP8, kind="Internal").ap()

    with tile.TileContext(nc) as tc:
        with (
            tc.tile_pool(name="const", bufs=1) as cpool,
            tc.tile_pool(name="xin", bufs=6) as xpool,
            tc.tile_pool(name="sq", bufs=5) as sqpool,
            tc.tile_pool(name="xr", bufs=3) as xrpool,
            tc.tile_pool(name="nt", bufs=2) as ntpool,
            tc.tile_pool(name="row", bufs=4) as rpool,
            tc.tile_pool(name="work", bufs=2) as wpool,
            tc.tile_pool(name="hbuf", bufs=2) as hpool,
            tc.tile_pool(name="psGC", bufs=3, space="PSUM") as psGC,
            tc.tile_pool(name="psS", bufs=1, space="PSUM") as psS,
        ):
            wg_sb = cpool.tile([128, KT // 2, OT, 2, 128], FP8, tag="wg")
            wc_sb = cpool.tile([128, KT // 2, OT, 2, 128], FP8, tag="wc")
            augg = cpool.tile([1, OT, 2, 128], FP8, tag="augg")
            augc = cpool.tile([1, OT, 2, 128], FP8, tag="augc")
            onx = cpool.tile([128, 2, 128], FP8, tag="onx")
            onq = cpool.tile([128, 2, 128], FP8, tag="onq")

            def load_consts():
                nc.scalar.dma_start(onx[:], onx_d)
                nc.scalar.dma_start(onq[:], onq_d)
                nc.scalar.dma_start(wg_sb[:], wg_d)
                nc.scalar.dma_start(wc_sb[:], wc_d)
                nc.scalar.dma_start(augg[:], augg_d)
                nc.scalar.dma_start(augc[:], augc_d)

            x8_t = [None] * N_CHUNKS
            xsq_t = [None] * N_CHUNKS
            xr_t = [None] * N_CHUNKS
            st_t = [None] * N_CHUNKS     # PSUM stats tiles
            augr_t = [None] * N_CHUNKS   # [1,2,512] fp8 rows
            h_prev = [None] * OT

            def load_x(i, split=False):
                xc = xpool.tile([128, KT, CHUNK], FP8, tag="x8")
                if split:
                    half = KT // 2
                    nc.sync.dma_start(xc[:, :half, :], x8_d[i, :, :half, :])
                    nc.sync.dma_start(xc[:, half:, :], x8_d[i, :, half:, :])
                else:
                    nc.sync.dma_start(xc[:], x8_d[i])
                x8_t[i] = xc

            def load_xr(i):
                xr = xrpool.tile([128, OT, CHUNK], FP16, tag="xr")
                nc.sync.dma_start(xr[:], xr_d[i])
                xr_t[i] = xr

            def squares(i):
                xc = x8_t[i]
                sq = sqpool.tile([128, KT, CHUNK], FP8, tag="xsq")
                with nc.allow_low_precision(reason="fp8 squares only feed the var sum"):
                    half = KT // 2
                    nc.scalar.activation(sq[:, :half, :], xc[:, :half, :], AF.Square)
                    nc.scalar.activation(sq[:, half:, :], xc[:, half:, :], AF.Square)
                xsq_t[i] = sq

            def stats(i):
                """st row0 = sum_h x8, row1 = sum_h x8^2 (one PSUM bank)."""
                xc, sq = x8_t[i], xsq_t[i]
                st = psS.tile([128, CHUNK], F32, tag="st")
                for kp in range(KT // 2):
                    nc.tensor.matmul(
                        st[:], onx[:], xc[:, 2 * kp : 2 * kp + 2, :],
                        start=(kp == 0), stop=False, perf_mode=DR,
                    )
                for kp in range(KT // 2):
                    nc.tensor.matmul(
                        st[:], onq[:], sq[:, 2 * kp : 2 * kp + 2, :],
                        start=False, stop=(kp == KT // 2 - 1), perf_mode=DR,
                    )
                st_t[i] = st

            stgrow_t = [None] * N_CHUNKS

            def stg_copy(i, to_dram=True):
                # PSUM stats rows (0: sumx, 32: sumsq) -> partition-0 SBUF
                # staging tiles (-> DRAM scratch for the gps newton path)
                pair, par = divmod(i, 2)
                stgx = ntpool.tile([1, CHUNK], F32, tag="stgx")
                stgq = ntpool.tile([1, CHUNK], F32, tag="stgq")
                nc.scalar.activation(stgx[:], st_t[i][0:1, :], AF.Copy)
                nc.scalar.activation(stgq[:], st_t[i][32:33, :], AF.Copy)
                stgrow_t[i] = (stgx, stgq)
                if to_dram:
                    nc.sync.dma_start(st_dram[pair, 2 * par : 2 * par + 1], stgx[:])
                    nc.sync.dma_start(st_dram[pair, 2 * par + 1 : 2 * par + 2], stgq[:])

            def newton_rows(pair, eng=None):
                """Prologue-only: Newton rstd in row layout on the idle DVE.

                No partition transposes, no DRAM hops on the aug path - each
                DMA-to-DMA dependency costs ~10.4us of semaphore latency, which
                dominates kernel startup."""
                vv = eng if eng is not None else nc.vector
                for par in range(2):
                    i = 2 * pair + par
                    stgx, stgq = stgrow_t[i]
                    sxr, sqr = stgx[:], stgq[:]
                    mu = ntpool.tile([1, CHUNK], F32, tag="rmu")
                    m2 = ntpool.tile([1, CHUNK], F32, tag="rm2")
                    v = ntpool.tile([1, CHUNK], F32, tag="rv")
                    y = ntpool.tile([1, CHUNK], F32, tag="ry")
                    t1 = ntpool.tile([1, CHUNK], F32, tag="rt1")
                    vv.tensor_scalar_mul(mu[:], sxr, 1.0 / H)
                    vv.tensor_mul(m2[:], mu[:], mu[:])
                    vv.tensor_scalar(t1[:], sqr, 1.0 / H, EPS, OP.mult, OP.add)
                    vv.tensor_sub(v[:], t1[:], m2[:])
                    vv.tensor_scalar(y[:], v[:], -0.5, 1.5, OP.mult, OP.add)
                    for _ in range(2):
                        vv.tensor_mul(t1[:], v[:], y[:])
                        vv.tensor_mul(t1[:], t1[:], y[:])
                        vv.tensor_scalar(t1[:], t1[:], -0.5, 1.5, OP.mult, OP.add)
                        vv.tensor_mul(y[:], y[:], t1[:])
                    rT = ntpool.tile([1, CHUNK], BF16, tag="rrT")
                    ar = rpool.tile([1, 2, CHUNK], FP8, tag="augr")
                    with nc.allow_low_precision(reason="rstd bf16 / aug rows fp8"):
                        vv.tensor_scalar_mul(rT[:], y[:], 1.0 / 32.0)
                        vv.tensor_scalar_mul(ar[0:1, 0, :], mu[:], 8.0)
                        vv.tensor_mul(ar[0:1, 1, :], v[:], y[:])
                    nc.sync.dma_start(rs_dram[pair, par : par + 1, :], rT[:])
                    augr_t[i] = ar

            def newton(pair):
                """rstd = 1/sqrt(var+eps) via 2 Newton steps, t-on-partitions."""
                # [4,512] rows (a=chunk, two=kind) -> two loads of [128, a, 4]
                src3 = st_dram[pair].rearrange("(a two) (p j) -> two p a j", two=2, p=128)
                sxT = ntpool.tile([128, 2, 4], F32, tag="sxT")
                sqT = ntpool.tile([128, 2, 4], F32, tag="sqT")
                nc.sync.dma_start(sxT[:], src3[0])
                nc.sync.dma_start(sqT[:], src3[1])
                sx = sxT[:]
                sq = sqT[:]
                mu = ntpool.tile([128, 2, 4], F32, tag="mu")
                m2 = ntpool.tile([128, 2, 4], F32, tag="m2")
                v = ntpool.tile([128, 2, 4], F32, tag="v")
                y = ntpool.tile([128, 2, 4], F32, tag="y")
                t1 = ntpool.tile([128, 2, 4], F32, tag="t1")
                g = nc.gpsimd
                g.tensor_scalar_mul(mu[:], sx, 1.0 / H)
                g.tensor_mul(m2[:], mu[:], mu[:])         # mu^2
                g.tensor_scalar(t1[:], sq, 1.0 / H, EPS, OP.mult, OP.add)
                g.tensor_sub(v[:], t1[:], m2[:])          # var + eps
                g.tensor_scalar(y[:], v[:], -0.5, 1.5, OP.mult, OP.add)
                for _ in range(2):
                    g.tensor_mul(t1[:], v[:], y[:])
                    g.tensor_mul(t1[:], t1[:], y[:])
                    g.tensor_scalar(t1[:], t1[:], -0.5, 1.5, OP.mult, OP.add)
                    g.tensor_mul(y[:], y[:], t1[:])
                rT = ntpool.tile([128, 2, 4], BF16, tag="rT")
                m8 = ntpool.tile([128, 2, 4], FP8, tag="m8")
                i8 = ntpool.tile([128, 2, 4], FP8, tag="i8")
                with nc.allow_low_precision(reason="rstd bf16 / aug rows fp8"):
                    g.tensor_scalar_mul(rT[:], y[:], 1.0 / 32.0)
                    g.tensor_scalar_mul(m8[:], mu[:], 8.0)
                    g.tensor_mul(i8[:], v[:], y[:])       # 1/rstd = (var+eps)*rstd
                nc.sync.dma_start(
                    rs_dram[pair].rearrange("a (p j) -> p a j", p=128), rT[:]
                )
                agr = ag_dram[pair].rearrange("a s (p j) -> s p a j", p=128)
                nc.sync.dma_start(agr[0], m8[:])
                nc.sync.dma_start(agr[1], i8[:])
                for par in range(2):
                    i = 2 * pair + par
                    ar = rpool.tile([1, 2, CHUNK], FP8, tag="augr")
                    nc.sync.dma_start(ar[:], ag_dram[pair, par].unsqueeze(0))
                    augr_t[i] = ar

            def gemm_mains(i, half, w_sb):
                """The 8 rstd-independent k-pair matmuls of one (gate, o-half)."""
                xc = x8_t[i]
                dst = psGC.tile([128, 2, CHUNK], F32, tag="pgc")
                for s in range(2):
                    o = 2 * half + s
                    for kp in range(KT // 2):
                        nc.tensor.matmul(
                            dst[:, s, :],
                            w_sb[:, kp, o],
                            xc[:, 2 * kp : 2 * kp + 2, :],
                            start=(kp == 0), stop=False, perf_mode=DR,
                            skip_group_check=True,
                        )
                return dst

            def gemm_aug_one(i, half, aug, dst):
                """Rank-1 LN-correction matmuls (need mu/invrstd rows)."""
                ar = augr_t[i]
                for s in range(2):
                    o = 2 * half + s
                    nc.tensor.matmul(
                        dst[:, s, :], aug[:, o], ar[:],
                        start=False, stop=True, perf_mode=DR,
                        skip_group_check=True,
                    )

            def chunk_body(i, pre=None):
                if i + 4 < N_CHUNKS:
                    load_x(i + 4)
                if i + 1 < N_CHUNKS:
                    load_xr(i + 1)

                if pre:
                    pg0, pc0, pg1 = pre
                else:
                    pg0 = gemm_mains(i, 0, wg_sb)
                    pc0 = gemm_mains(i, 0, wc_sb)
                # rstd/32 broadcast via stride-0 DMA from Newton's DRAM row
                pair, par = divmod(i, 2)
                psbS = wpool.tile([128, CHUNK], BF16, tag="psbS")
                nc.scalar.dma_start(
                    psbS[:],
                    rs_dram[pair, par].unsqueeze(0).broadcast_to([128, CHUNK]),
                )
                gemm_aug_one(i, 0, augg, pg0)
                gemm_aug_one(i, 0, augc, pc0)
                if i + 4 < N_CHUNKS:
                    squares(i + 4)      # ACT fills while PE runs GEMMs
                if not pre:
                    pg1 = gemm_mains(i, 1, wg_sb)
                if i + 4 < N_CHUNKS:
                    stats(i + 4)        # PE mid-chunk
                pc1 = gemm_mains(i, 1, wc_sb)
                gemm_aug_one(i, 1, augg, pg1)
                gemm_aug_one(i, 1, augc, pc1)

                with nc.allow_low_precision(reason="bf16 gate/candidate path"):
                    tg = wpool.tile([128, OT, CHUNK], BF16, tag="tg")
                    tc_ = wpool.tile([128, OT, CHUNK], BF16, tag="tcn")
                    z = wpool.tile([128, OT, CHUNK], BF16, tag="z")
                    zb = wpool.tile([128, OT, CHUNK], BF16, tag="zb")
                    bsc = wpool.tile([128, OT, CHUNK], BF16, tag="bsc")
                    h = hpool.tile([128, OT, CHUNK], BF16, tag="h")
                    for half, (pg, pc) in ((0, (pg0, pc0)), (1, (pg1, pc1))):
                        sl = slice(2 * half, 2 * half + 2)
                        for s in range(2):
                            o = 2 * half + s
                            nc.vector.tensor_mul(tg[:, o, :], pg[:, s, :], psbS[:])
                            nc.vector.tensor_mul(tc_[:, o, :], pc[:, s, :], psbS[:])
                        nc.scalar.activation(z[:, sl, :], tg[:, sl, :], AF.Sigmoid)
                        nc.vector.tensor_scalar(
                            zb[:, sl, :], z[:, sl, :], -1.0, 1.0, OP.mult, OP.add
                        )
                        nc.vector.tensor_mul(bsc[:, sl, :], tc_[:, sl, :], z[:, sl, :])
                        for s in range(2):
                            o = 2 * half + s
                            init = 0.0 if i == 0 else h_prev[o][:, CHUNK - 1 : CHUNK]
                            nc.vector.tensor_tensor_scan(
                                h[:, o, :], zb[:, o, :], bsc[:, o, :], init,
                                OP.mult, OP.add,
                            )
                            h_prev[o] = h[:, o, :]
                        if half == 0:
                            if i + 4 < N_CHUNKS:
                                stg_copy(i + 4)
                            if i % 2 == 1 and (i + 3) // 2 < NPAIR:
                                newton((i + 3) // 2)
                    ot = wpool.tile([128, OT, CHUNK], FP16, tag="ot")
                    nc.gpsimd.tensor_add(ot[:], h[:], xr_t[i][:])
                nc.gpsimd.dma_start(out_d[i].transpose([1, 0, 2]), ot[:])

            # ---- prologue: chunk-0 mains keep the PE hot while the stats
            # chain (squares/stats/newton) for chunks 0-2 runs on ACT/GpSimd ----
            load_x(0, split=True)
            load_consts()
            load_xr(0)
            load_x(1)
            load_x(2)
            load_x(3)
            # pin the ACT table to the sigmoid set (covers square/copy too)
            with nc.allow_low_precision(reason="dummy table-pin op"):
                tpin = ntpool.tile([1, 16], BF16, tag="tpin")
                nc.scalar.activation(tpin[:], onx[0:1, 0, 0:16], AF.Sigmoid)
            pre_g0 = gemm_mains(0, 0, wg_sb)
            squares(0)
            stats(0)
            stg_copy(0, to_dram=False)
            pre_c0 = gemm_mains(0, 0, wc_sb)
            squares(1)
            stats(1)
            stg_copy(1, to_dram=False)
            newton_rows(0)
            pre_g1 = gemm_mains(0, 1, wg_sb)
            squares(2)
            squares(3)
            stats(2)
            stats(3)
            stg_copy(2, to_dram=False)
            stg_copy(3, to_dram=False)
            newton_rows(1, eng=nc.gpsimd)
            chunk_body(0, pre=(pre_g0, pre_c0, pre_g1))
            for i in range(1, N_CHUNKS):
                chunk_body(i)

    nc.compile()
    return nc


def _prep_weights(gamma, beta, Wg, bg, Wc, bc, ohalf):
    """Host-side weight folding for one output half (fp8, 32x scaled)."""
    o0 = ohalf * OH
    perm = np.roll(np.arange(H), -o0)
    out = {}
    for nm, W, b in (("g", Wg, bg), ("c", Wc, bc)):
        W_h = W[o0 : o0 + OH]                                   # [OH, H]
        w_eff = ((W_h * gamma[None, :]).T)[perm]                # [H, OH]
        b_eff = b[o0 : o0 + OH] + W_h @ beta                    # [OH]
        w8 = (32.0 * w_eff).astype(F8)                          # [H, OH] fp8
        wsumq = w8.astype(np.float32).sum(axis=0) / 32.0        # [OH]
        aug = np.zeros((1, 2, OH), dtype=F8)
        aug[0, 0] = (-4.0 * wsumq).astype(F8)
        aug[0, 1] = (32.0 * b_eff).astype(F8)
        out["w" + nm] = np.ascontiguousarray(
            w8.reshape(KT // 2, 2, 128, OT, 128).transpose(2, 0, 3, 1, 4)
        )
        out["aug" + nm] = np.ascontiguousarray(
            aug.reshape(1, 2, OT, 128).transpose(0, 2, 1, 3)
        )
    onx = np.zeros((128, 2, 128), dtype=F8)
    onx[:, :, 0] = 1.0
    onq = np.zeros((128, 2, 128), dtype=F8)
    onq[:, :, 32] = 1.0
    out["onx"] = onx
    out["onq"] = onq
    return out


def kernel(x, gamma, beta, Wg, bg, Wc, bc):
    x = np.asarray(x, dtype=np.float32)
    gamma = np.asarray(gamma, dtype=np.float32)
    beta = np.asarray(beta, dtype=np.float32)
    Wg = np.asarray(Wg, dtype=np.float32)
    bg = np.asarray(bg, dtype=np.float32)
    Wc = np.asarray(Wc, dtype=np.float32)
    bc = np.asarray(bc, dtype=np.float32)

    if "nc" not in _CACHE:
        _CACHE["nc"] = _build()
    nc = _CACHE["nc"]

    xT = [np.ascontiguousarray(x[b].T) for b in range(B)]  # [H, T] each
    halves = [_prep_weights(gamma, beta, Wg, bg, Wc, bc, p) for p in range(2)]

    in_maps = []
    for c in range(N_CORES):
        b, p = divmod(c, 2)
        m = dict(halves[p])
        xr = xT[b] if p == 0 else np.roll(xT[b], -OH, axis=0)
        m["x8"] = np.ascontiguousarray(
            xr.astype(F8).reshape(KT, 128, N_CHUNKS, CHUNK).transpose(2, 1, 0, 3)
        )
        m["xr"] = np.ascontiguousarray(
            xr[:OH].astype(np.float16).reshape(OT, 128, N_CHUNKS, CHUNK).transpose(2, 1, 0, 3)
        )
        in_maps.append(m)

    trace = bool(int(os.environ.get("MINGRU_TRACE", "0")))
    kwargs = {}
    if trace:
        tmpdir = os.environ.get("MINGRU_TRACE_DIR") or None
        kwargs = dict(trace=True, tmpdir=tmpdir)
    res = run_bass_kernel_spmd(nc, in_maps, core_ids=list(range(N_CORES)), **kwargs)
    if trace:
        _CACHE["last_results"] = res

    out = np.empty((B, T, H), dtype=np.float32)
    for c in range(N_CORES):
        b, p = divmod(c, 2)
        oT = res.results[c]["outT"].astype(np.float32).transpose(1, 2, 0, 3).reshape(OH, T)
        out[b, :, p * OH : (p + 1) * OH] = oT.T
    return out


# revision 33
# speedup vs baseline: 1.2444x; 1.2444x over previous
"""MinGRU layer (LN -> gate/candidate Linear -> minGRU scan -> residual) on 8 trn2 cores.

Problem (hardcoded): x [B=4, T=4096, H=1024] fp32, weights Wg/Wc [1024,1024],
biases bg/bc [1024], LN gamma/beta [1024].

Sharding: core c = (batch b = c//2, output-half p = c%2). Each core receives the
full transposed batch row (H on partitions, T on free) and computes z/c for its
512 output channels over all T; the minGRU recurrence is elementwise over (b,h)
so no collectives are needed.

Per 512-col chunk, built around fp8 DoubleRow matmuls (each streams a PAIR of
k-tiles -> 2x bf16 GEMM throughput; weights pre-scaled by 32 on host so fp8
stays in normal range):
  1. GEMM on RAW fp8 x (no prescale): P' = 32W.x8 over 4 k-pair DR matmuls plus
     one rank-1 aug DR matmul carrying both LayerNorm corrections:
     slot0 = (-4*wsumq) x (8*mu), slot1 = (32*b_eff) x (1/rstd).  Then
     tmp = P' * (rstd/32 broadcast) = W.((x-mu)*rstd) + b exactly (post-scale
     keeps P' in fp32 PSUM - no fp8 requantize of the scaled activations).
  2. Stats by the same DR trick: ones-weights (col 0 / col 1) sum x8 and x8^2
     pairs into rows 0/1 of one PSUM bank; squares x8^2 on ScalarE.
  3. rstd via 2 Newton steps for 1/sqrt(var+eps) on GpSimd in a DMA-transposed
     [128, 2, 4] layout (t on partitions, both chunks of a pair at once) - no
     Ln/Exp, so the ACT table never leaves the sigmoid/square/copy set.
  4. z = sigmoid(tmp_g) on ScalarE (bias pre-folded via aug slot1); zbar = 1-z
     (VectorE 4x); bsc = tmp_c*z (GpSimd); h = scan(zbar, bsc) (VectorE, the
     only engine with tensor_tensor_scan); residual h + x(fp16) on VectorE in
     2x mode; fp16 output, host transposes back.
"""

import os
import numpy as np
import ml_dtypes

import concourse.bass as bass
import concourse.bacc as bacc
import concourse.tile as tile
from concourse import mybir
from concourse.bass_utils import run_bass_kernel_spmd

B, T, H = 4, 4096, 1024
EPS = 1e-5
N_CORES = 8
OH = H // 2          # output channels per core
CHUNK = 512
N_CHUNKS = T // CHUNK
KT = H // 128        # k-tiles (contraction)
OT = OH // 128       # o-tiles per core
NPAIR = N_CHUNKS // 2

F32 = mybir.dt.float32
BF16 = mybir.dt.bfloat16
FP16 = mybir.dt.float16
FP8 = mybir.dt.float8e4
AF = mybir.ActivationFunctionType
OP = mybir.AluOpType
DR = mybir.MatmulPerfMode.DoubleRow
BF = ml_dtypes.bfloat16
F8 = ml_dtypes.float8_e4m3

_CACHE = {}


def _build():
    nc = bacc.Bacc("TRN2", target_bir_lowering=False, debug=False)

    x8_d = nc.dram_tensor("x8", [N_CHUNKS, 128, KT, CHUNK], FP8, kind="ExternalInput").ap()
    xr_d = nc.dram_tensor("xr", [N_CHUNKS, 128, OT, CHUNK], FP16, kind="ExternalInput").ap()
    wg_d = nc.dram_tensor("wg", [128, KT // 2, OT, 2, 128], FP8, kind="ExternalInput").ap()
    wc_d = nc.dram_tensor("wc", [128, KT // 2, OT, 2, 128], FP8, kind="ExternalInput").ap()
    augg_d = nc.dram_tensor("augg", [1, OT, 2, 128], FP8, kind="ExternalInput").ap()
    augc_d = nc.dram_tensor("augc", [1, OT, 2, 128], FP8, kind="ExternalInput").ap()
    onx_d = nc.dram_tensor("onx", [128, 2, 128], FP8, kind="ExternalInput").ap()
    onq_d = nc.dram_tensor("onq", [128, 2, 128], FP8, kind="ExternalInput").ap()
    out_d = nc.dram_tensor("outT", [N_CHUNKS, OT, 128, CHUNK], FP16, kind="ExternalOutput").ap()

    # Newton scratch in DRAM (partition-crossing transposes go through HBM).
    # stg rows per pair: (sx0, sq0, sx1, sq1)
    st_dram = nc.dram_tensor("st_sc", [NPAIR, 4, CHUNK], F32, kind="Internal").ap()
    rs_dram = nc.dram_tensor("rs_sc", [NPAIR, 2, CHUNK], BF16, kind="Internal").ap()
    ag_dram = nc.dram_tensor("ag_sc", [NPAIR, 2, 2, CHUNK], FP8, kind="Internal").ap()

    with tile.TileContext(nc) as tc:
        with (
            tc.tile_pool(name="const", bufs=1) as cpool,
            tc.tile_pool(name="xin", bufs=6) as xpool,
            tc.tile_pool(name="sq", bufs=5) as sqpool,
            tc.tile_pool(name="xr", bufs=3) as xrpool,
            tc.tile_pool(name="nt", bufs=2) as ntpool,
            tc.tile_pool(name="row", bufs=4) as rpool,
            tc.tile_pool(name="work", bufs=2) as wpool,
            tc.tile_pool(name="hbuf", bufs=2) as hpool,
            tc.tile_pool(name="psGC", bufs=3, space="PSUM") as psGC,
            tc.tile_pool(name="psS", bufs=1, space="PSUM") as psS,
        ):
            wg_sb = cpool.tile([128, KT // 2, OT, 2, 128], FP8, tag="wg")
            wc_sb = cpool.tile([128, KT // 2, OT, 2, 128], FP8, tag="wc")
            augg = cpool.tile([1, OT, 2, 128], FP8, tag="augg")
            augc = cpool.tile([1, OT, 2, 128], FP8, tag="augc")
            onx = cpool.tile([128, 2, 128], FP8, tag="onx")
            onq = cpool.tile([128, 2, 128], FP8, tag="onq")

            def load_consts():
                nc.scalar.dma_start(onx[:], onx_d)
                nc.scalar.dma_start(onq[:], onq_d)
                nc.scalar.dma_start(wg_sb[:], wg_d)
                nc.scalar.dma_start(wc_sb[:], wc_d)
                nc.scalar.dma_start(augg[:], augg_d)
                nc.scalar.dma_start(augc[:], augc_d)

            x8_t = [None] * N_CHUNKS
            xsq_t = [None] * N_CHUNKS
            xr_t = [None] * N_CHUNKS
            st_t = [None] * N_CHUNKS     # PSUM stats tiles
            augr_t = [None] * N_CHUNKS   # [1,2,512] fp8 rows
            h_prev = [None] * OT

            def load_x(i, split=False):
                xc = xpool.tile([128, KT, CHUNK], FP8, tag="x8")
                if split:
                    half = KT // 2
                    nc.sync.dma_start(xc[:, :half, :], x8_d[i, :, :half, :])
                    nc.sync.dma_start(xc[:, half:, :], x8_d[i, :, half:, :])
                else:
                    nc.sync.dma_start(xc[:], x8_d[i])
                x8_t[i] = xc

            def load_xr(i):
                xr = xrpool.tile([128, OT, CHUNK], FP16, tag="xr")
                nc.sync.dma_start(xr[:], xr_d[i])
                xr_t[i] = xr

            def squares(i):
                xc = x8_t[i]
                sq = sqpool.tile([128, KT, CHUNK], FP8, tag="xsq")
                with nc.allow_low_precision(reason="fp8 squares only feed the var sum"):
                    half = KT // 2
                    nc.scalar.activation(sq[:, :half, :], xc[:, :half, :], AF.Square)
                    nc.scalar.activation(sq[:, half:, :], xc[:, half:, :], AF.Square)
                xsq_t[i] = sq

            def stats(i):
                """st row0 = sum_h x8, row1 = sum_h x8^2 (one PSUM bank)."""
                xc, sq = x8_t[i], xsq_t[i]
                st = psS.tile([128, CHUNK], F32, tag="st")
                for kp in range(KT // 2):
                    nc.tensor.matmul(
                        st[:], onx[:], xc[:, 2 * kp : 2 * kp + 2, :],
                        start=(kp == 0), stop=False, perf_mode=DR,
                    )
                for kp in range(KT // 2):
                    nc.tensor.matmul(
                        st[:], onq[:], sq[:, 2 * kp : 2 * kp + 2, :],
                        start=False, stop=(kp == KT // 2 - 1), perf_mode=DR,
                    )
                st_t[i] = st

            stgrow_t = [None] * N_CHUNKS

            def stg_copy(i, to_dram=True):
                # PSUM stats rows (0: sumx, 32: sumsq) -> partition-0 SBUF
                # staging tiles (-> DRAM scratch for the gps newton path)
                pair, par = divmod(i, 2)
                stgx = ntpool.tile([1, CHUNK], F32, tag="stgx")
                stgq = ntpool.tile([1, CHUNK], F32, tag="stgq")
                nc.scalar.activation(stgx[:], st_t[i][0:1, :], AF.Copy)
                nc.scalar.activation(stgq[:], st_t[i][32:33, :], AF.Copy)
                stgrow_t[i] = (stgx, stgq)
                if to_dram:
                    nc.sync.dma_start(st_dram[pair, 2 * par : 2 * par + 1], stgx[:])
                    nc.sync.dma_start(st_dram[pair, 2 * par + 1 : 2 * par + 2], stgq[:])

            def newton_rows(pair):
                """Prologue-only: Newton rstd in row layout on the idle DVE.

                No partition transposes, no DRAM hops on the aug path - each
                DMA-to-DMA dependency costs ~10.4us of semaphore latency, which
                dominates kernel startup."""
                vv = nc.vector
                for par in range(2):
                    i = 2 * pair + par
                    stgx, stgq = stgrow_t[i]
                    sxr, sqr = stgx[:], stgq[:]
                    mu = ntpool.tile([1, CHUNK], F32, tag="rmu")
                    m2 = ntpool.tile([1, CHUNK], F32, tag="rm2")
                    v = ntpool.tile([1, CHUNK], F32, tag="rv")
                    y = ntpool.tile([1, CHUNK], F32, tag="ry")
                    t1 = ntpool.tile([1, CHUNK], F32, tag="rt1")
                    vv.tensor_scalar_mul(mu[:], sxr, 1.0 / H)
                    vv.tensor_mul(m2[:], mu[:], mu[:])
                    vv.tensor_scalar(t1[:], sqr, 1.0 / H, EPS, OP.mult, OP.add)
                    vv.tensor_sub(v[:], t1[:], m2[:])
                    vv.tensor_scalar(y[:], v[:], -0.5, 1.5, OP.mult, OP.add)
                    for _ in range(2):
                        vv.tensor_mul(t1[:], v[:], y[:])
                        vv.tensor_mul(t1[:], t1[:], y[:])
                        vv.tensor_scalar(t1[:], t1[:], -0.5, 1.5, OP.mult, OP.add)
                        vv.tensor_mul(y[:], y[:], t1[:])
                    rT = ntpool.tile([1, CHUNK], BF16, tag="rrT")
                    ar = rpool.tile([1, 2, CHUNK], FP8, tag="augr")
                    with nc.allow_low_precision(reason="rstd bf16 / aug rows fp8"):
                        vv.tensor_scalar_mul(rT[:], y[:], 1.0 / 32.0)
                        vv.tensor_scalar_mul(ar[0:1, 0, :], mu[:], 8.0)
                        vv.tensor_mul(ar[0:1, 1, :], v[:], y[:])
                    nc.sync.dma_start(rs_dram[pair, par : par + 1, :], rT[:])
                    augr_t[i] = ar

            def newton(pair):
                """rstd = 1/sqrt(var+eps) via 2 Newton steps, t-on-partitions."""
                # [4,512] rows (a=chunk, two=kind) -> two loads of [128, a, 4]
                src3 = st_dram[pair].rearrange("(a two) (p j) -> two p a j", two=2, p=128)
                sxT = ntpool.tile([128, 2, 4], F32, tag="sxT")
                sqT = ntpool.tile([128, 2, 4], F32, tag="sqT")
                nc.sync.dma_start(sxT[:], src3[0])
                nc.sync.dma_start(sqT[:], src3[1])
                sx = sxT[:]
                sq = sqT[:]
                mu = ntpool.tile([128, 2, 4], F32, tag="mu")
                m2 = ntpool.tile([128, 2, 4], F32, tag="m2")
                v = ntpool.tile([128, 2, 4], F32, tag="v")
                y = ntpool.tile([128, 2, 4], F32, tag="y")
                t1 = ntpool.tile([128, 2, 4], F32, tag="t1")
                g = nc.gpsimd
                g.tensor_scalar_mul(mu[:], sx, 1.0 / H)
                g.tensor_mul(m2[:], mu[:], mu[:])         # mu^2
                g.tensor_scalar(t1[:], sq, 1.0 / H, EPS, OP.mult, OP.add)
                g.tensor_sub(v[:], t1[:], m2[:])          # var + eps
                g.tensor_scalar(y[:], v[:], -0.5, 1.5, OP.mult, OP.add)
                for _ in range(2):
                    g.tensor_mul(t1[:], v[:], y[:])
                    g.tensor_mul(t1[:], t1[:], y[:])
                    g.tensor_scalar(t1[:], t1[:], -0.5, 1.5, OP.mult, OP.add)
                    g.tensor_mul(y[:], y[:], t1[:])
                rT = ntpool.tile([128, 2, 4], BF16, tag="rT")
                m8 = ntpool.tile([128, 2, 4], FP8, tag="m8")
                i8 = ntpool.tile([128, 2, 4], FP8, tag="i8")
                with nc.allow_low_precision(reason="rstd bf16 / aug rows fp8"):
                    g.tensor_scalar_mul(rT[:], y[:], 1.0 / 32.0)
                    g.tensor_scalar_mul(m8[:], mu[:], 8.0)
                    g.tensor_mul(i8[:], v[:], y[:])       # 1/rstd = (var+eps)*rstd
                nc.sync.dma_start(
                    rs_dram[pair].rearrange("a (p j) -> p a j", p=128), rT[:]
                )
                agr = ag_dram[pair].rearrange("a s (p j) -> s p a j", p=128)
                nc.sync.dma_start(agr[0], m8[:])
                nc.sync.dma_start(agr[1], i8[:])
                for par in range(2):
                    i = 2 * pair + par
                    ar = rpool.tile([1, 2, CHUNK], FP8, tag="augr")
                    nc.sync.dma_start(ar[:], ag_dram[pair, par].unsqueeze(0))
                    augr_t[i] = ar

            def gemm_mains(i, half, w_sb):
                """The 8 rstd-independent k-pair matmuls of one (gate, o-half)."""
                xc = x8_t[i]
                dst = psGC.tile([128, 2, CHUNK], F32, tag="pgc")
                for s in range(2):
                    o = 2 * half + s
                    for kp in range(KT // 2):
                        nc.tensor.matmul(
                            dst[:, s, :],
                            w_sb[:, kp, o],
                            xc[:, 2 * kp : 2 * kp + 2, :],
                            start=(kp == 0), stop=False, perf_mode=DR,
                            skip_group_check=True,
                        )
                return dst

            def gemm_aug_one(i, half, aug, dst):
                """Rank-1 LN-correction matmuls (need mu/invrstd rows)."""
                ar = augr_t[i]
                for s in range(2):
                    o = 2 * half + s
                    nc.tensor.matmul(
                        dst[:, s, :], aug[:, o], ar[:],
                        start=False, stop=True, perf_mode=DR,
                        skip_group_check=True,
                    )

            def chunk_body(i, pre=None):
                if i + 4 < N_CHUNKS:
                    load_x(i + 4)
                if i + 1 < N_CHUNKS:
                    load_xr(i + 1)

                if pre:
                    pg0, pc0, pg1 = pre
                else:
                    pg0 = gemm_mains(i, 0, wg_sb)
                    pc0 = gemm_mains(i, 0, wc_sb)
                # rstd/32 broadcast via stride-0 DMA from Newton's DRAM row
                pair, par = divmod(i, 2)
                psbS = wpool.tile([128, CHUNK], BF16, tag="psbS")
                nc.scalar.dma_start(
                    psbS[:],
                    rs_dram[pair, par].unsqueeze(0).broadcast_to([128, CHUNK]),
                )
                gemm_aug_one(i, 0, augg, pg0)
                gemm_aug_one(i, 0, augc, pc0)
                if i + 4 < N_CHUNKS:
                    squares(i + 4)      # ACT fills while PE runs GEMMs
                if not pre:
                    pg1 = gemm_mains(i, 1, wg_sb)
                if i + 4 < N_CHUNKS:
                    stats(i + 4)        # PE mid-chunk
                pc1 = gemm_mains(i, 1, wc_sb)
                gemm_aug_one(i, 1, augg, pg1)
                gemm_aug_one(i, 1, augc, pc1)

                with nc.allow_low_precision(reason="bf16 gate/candidate path"):
                    tg = wpool.tile([128, OT, CHUNK], BF16, tag="tg")
                    tc_ = wpool.tile([128, OT, CHUNK], BF16, tag="tcn")
                    z = wpool.tile([128, OT, CHUNK], BF16, tag="z")
                    zb = wpool.tile([128, OT, CHUNK], BF16, tag="zb")
                    bsc = wpool.tile([128, OT, CHUNK], BF16, tag="bsc")
                    h = hpool.tile([128, OT, CHUNK], BF16, tag="h")
                    for half, (pg, pc) in ((0, (pg0, pc0)), (1, (pg1, pc1))):
                        sl = slice(2 * half, 2 * half + 2)
                        for s in range(2):
                            o = 2 * half + s
                            nc.vector.tensor_mul(tg[:, o, :], pg[:, s, :], psbS[:])
                            nc.vector.tensor_mul(tc_[:, o, :], pc[:, s, :], psbS[:])
                        nc.scalar.activation(z[:, sl, :], tg[:, sl, :], AF.Sigmoid)
                        nc.vector.tensor_scalar(
                            zb[:, sl, :], z[:, sl, :], -1.0, 1.0, OP.mult, OP.add
                        )
                        nc.vector.tensor_mul(bsc[:, sl, :], tc_[:, sl, :], z[:, sl, :])
                        for s in range(2):
                            o = 2 * half + s
                            init = 0.0 if i == 0 else h_prev[o][:, CHUNK - 1 : CHUNK]
                            nc.vector.tensor_tensor_scan(
                                h[:, o, :], zb[:, o, :], bsc[:, o, :], init,
                                OP.mult, OP.add,
                            )
                            h_prev[o] = h[:, o, :]
                        if half == 0:
                            if i + 4 < N_CHUNKS:
                                stg_copy(i + 4)
                            if i % 2 == 1 and (i + 3) // 2 < NPAIR:
                                newton((i + 3) // 2)
                    ot = wpool.tile([128, OT, CHUNK], FP16, tag="ot")
                    nc.gpsimd.tensor_add(ot[:], h[:], xr_t[i][:])
                nc.gpsimd.dma_start(out_d[i].transpose([1, 0, 2]), ot[:])

            # ---- prologue: chunk-0 mains keep the PE hot while the stats
            # chain (squares/stats/newton) for chunks 0-2 runs on ACT/GpSimd ----
            load_x(0, split=True)
            load_consts()
            load_xr(0)
            load_x(1)
            load_x(2)
            load_x(3)
            # pin the ACT table to the sigmoid set (covers square/copy too)
            with nc.allow_low_precision(reason="dummy table-pin op"):
                tpin = ntpool.tile([1, 16], BF16, tag="tpin")
                nc.scalar.activation(tpin[:], onx[0:1, 0, 0:16], AF.Sigmoid)
            pre_g0 = gemm_mains(0, 0, wg_sb)
            squares(0)
            stats(0)
            stg_copy(0, to_dram=False)
            pre_c0 = gemm_mains(0, 0, wc_sb)
            squares(1)
            stats(1)
            stg_copy(1, to_dram=False)
            newton_rows(0)
            pre_g1 = gemm_mains(0, 1, wg_sb)
            squares(2)
            squares(3)
            stats(2)
            stats(3)
            stg_copy(2)
            stg_copy(3)
            newton(1)
            chunk_body(0, pre=(pre_g0, pre_c0, pre_g1))
            for i in range(1, N_CHUNKS):
                chunk_body(i)

    nc.compile()
    return nc


def _prep_weights(gamma, beta, Wg, bg, Wc, bc, ohalf):
    """Host-side weight folding for one output half (fp8, 32x scaled)."""
    o0 = ohalf * OH
    perm = np.roll(np.arange(H), -o0)
    out = {}
    for nm, W, b in (("g", Wg, bg), ("c", Wc, bc)):
        W_h = W[o0 : o0 + OH]                                   # [OH, H]
        w_eff = ((W_h * gamma[None, :]).T)[perm]                # [H, OH]
        b_eff = b[o0 : o0 + OH] + W_h @ beta                    # [OH]
        w8 = (32.0 * w_eff).astype(F8)                          # [H, OH] fp8
        wsumq = w8.astype(np.float32).sum(axis=0) / 32.0        # [OH]
        aug = np.zeros((1, 2, OH), dtype=F8)
        aug[0, 0] = (-4.0 * wsumq).astype(F8)
        aug[0, 1] = (32.0 * b_eff).astype(F8)
        out["w" + nm] = np.ascontiguousarray(
            w8.reshape(KT // 2, 2, 128, OT, 128).transpose(2, 0, 3, 1, 4)
        )
        out["aug" + nm] = np.ascontiguousarray(
            aug.reshape(1, 2, OT, 128).transpose(0, 2, 1, 3)
        )
    onx = np.zeros((128, 2, 128), dtype=F8)
    onx[:, :, 0] = 1.0
    onq = np.zeros((128, 2, 128), dtype=F8)
    onq[:, :, 32] = 1.0
    out["onx"] = onx
    out["onq"] = onq
    return out


def kernel(x, gamma, beta, Wg, bg, Wc, bc):
    x = np.asarray(x, dtype=np.float32)
    gamma = np.asarray(gamma, dtype=np.float32)
    beta = np.asarray(beta, dtype=np.float32)
    Wg = np.asarray(Wg, dtype=np.float32)
    bg = np.asarray(bg, dtype=np.float32)
    Wc = np.asarray(Wc, dtype=np.float32)
    bc = np.asarray(bc, dtype=np.float32)

    if "nc" not in _CACHE:
        _CACHE["nc"] = _build()
    nc = _CACHE["nc"]

    xT = [np.ascontiguousarray(x[b].T) for b in range(B)]  # [H, T] each
    halves = [_prep_weights(gamma, beta, Wg, bg, Wc, bc, p) for p in range(2)]

    in_maps = []
    for c in range(N_CORES):
        b, p = divmod(c, 2)
        m = dict(halves[p])
        xr = xT[b] if p == 0 else np.roll(xT[b], -OH, axis=0)
        m["x8"] = np.ascontiguousarray(
            xr.astype(F8).reshape(KT, 128, N_CHUNKS, CHUNK).transpose(2, 1, 0, 3)
        )
        m["xr"] = np.ascontiguousarray(
            xr[:OH].astype(np.float16).reshape(OT, 128, N_CHUNKS, CHUNK).transpose(2, 1, 0, 3)
        )
        in_maps.append(m)

    trace = bool(int(os.environ.get("MINGRU_TRACE", "0")))
    kwargs = {}
    if trace:
        tmpdir = os.environ.get("MINGRU_TRACE_DIR") or None
        kwargs = dict(trace=True, tmpdir=tmpdir)
    res = run_bass_kernel_spmd(nc, in_maps, core_ids=list(range(N_CORES)), **kwargs)
    if trace:
        _CACHE["last_results"] = res

    out = np.empty((B, T, H), dtype=np.float32)
    for c in range(N_CORES):
        b, p = divmod(c, 2)
        oT = res.results[c]["outT"].astype(np.float32).transpose(1, 2, 0, 3).reshape(OH, T)
        out[b, :, p * OH : (p + 1) * OH] = oT.T
    return out
